# revision 60
# baseline (speedup 1.0000x reference)
# Additive (Bahdanau) attention Trainium2 kernel.
#
# Problem shapes (hardcoded): B=4, Tq=256, Tv=1024, D=512, A=128.
#   k = inputs @ Wk + bk                  [B,Tv,A]
#   q = context @ Wq + bq                 [B,Tq,A]
#   scores[b,i,v] = sum_a attn_v[a] * tanh(q[b,i,a] + k[b,v,a]) + (1-mask)*NEG_BIG
#   out = softmax_v(scores) @ inputs      [B,Tq,D]
#
# Sharding: 8 cores = (batch b = c//2) x (query half qh = c%2); each core owns
# 128 queries with the full Tv, so softmax is local and no collectives are
# needed.
#
# Per-core dataflow (ACT/tanh-bound; other engines hide under it):
#   PE:  transpose inputs/context -> kT[a,v], qb[a,q] projections (float32r)
#   DVE: S[a, (j,v)] = kT[a,v] + qb[a,q]       (tensor_scalar, 2x mode)
#   ACT: T = tanh(S) on G-query batches        (the 16.8M-element bottleneck)
#   PE:  scores[q,v] accumulated with shifted one-hot weight columns so each
#        query's weighted A-reduction lands on its own PSUM partition
#   softmax: raw exp (scores bounded by ||attn_v||_1 ~ 9.2) + accum_out sum
#   PE:  transpose exp(P) -> P^T; out = P^T.T @ inputs accumulated per
#        d-half so the first half's scale+store overlaps the second half;
#        scale by 1/sumexp
#
# The engines execute their instruction streams in order, so emission order
# below is hand-interleaved: input v-blocks flow DMA -> PE transpose -> PSUM
# evac (alternating ACT/DVE) -> k-projection per Tv-half -> DVE preadds, so
# the ACT tanh stream starts ~13us in and then runs gap-free to the end.
# Dependency notes baked into the structure:
#  - Tile tracks dependencies per-tile, not per-range: kT / inpT / scores /
#    expP / pT are split into per-half tiles so half-0 consumers never wait
#    on half-1 producers.
#  - fp32 matmuls stream at 4 cycles/row; float32r (same bytes, tf32-like
#    multiply, fp32 accumulate) streams at 1 cycle/row for free dims >= 256,
#    so every large matmul operand is float32r. Measured output error vs the
#    fp32 reference: ~2.4e-4 max relative.
#  - The first NPRE groups and the last group are emitted h-major (all
#    half-0 work, then half-1) to hide the second input half's DMA at the
#    start and to overlap the softmax/output chain with the last tanh.

import time

import numpy as np

import concourse.bass as bass
import concourse.tile as tile
from concourse import bacc, mybir
from concourse import bass_utils
from concourse.masks import make_identity

P = 128
B, Tq, Tv, D, A = 4, 256, 1024, 512, 128
NCORES = 8
QC = Tq // 2          # queries per core
DC = D // P           # d chunks (4)
VB = Tv // P          # v blocks (8)
G = 4                 # queries per tanh batch
NG = QC // G          # groups (32)
NPRE = 3              # pipeline-prefill groups, emitted per-half
NEG_BIG = -1e9

F32 = mybir.dt.float32
F32R = mybir.dt.float32r
I32 = mybir.dt.int32
AF = mybir.ActivationFunctionType


def build_nc():
    nc = bacc.Bacc("TRN2", target_bir_lowering=False, debug=False)

    inp_d = nc.dram_tensor("inp", (Tv, D), F32R, kind="ExternalInput")
    ctx_d = nc.dram_tensor("ctx", (QC, D), F32R, kind="ExternalInput")
    msk_d = nc.dram_tensor("mask", (1, Tv), I32, kind="ExternalInput")
    wkq_d = nc.dram_tensor("wkq", (D, 2 * A), F32R, kind="ExternalInput")
    bba_d = nc.dram_tensor("bba", (A, 3), F32, kind="ExternalInput")
    y_d = nc.dram_tensor("y", (QC, D), F32, kind="ExternalOutput")

    with tile.TileContext(nc) as tc:
        with (
            tc.tile_pool(name="const", bufs=1) as const,
            tc.tile_pool(name="spool", bufs=3) as spool,
            tc.tile_pool(name="tpool", bufs=3) as tpool,
            tc.tile_pool(name="ps_tr", bufs=4, space="PSUM") as ps_tr,
            tc.tile_pool(name="ps_proj", bufs=2, space="PSUM") as ps_proj,
            tc.tile_pool(name="ps_sc", bufs=1, space="PSUM") as ps_sc,
        ):
            # ---- loads (DMA issue overhead ~650ns each; count minimized,
            # ordered so the first input half lands as early as possible) ----
            wkq_sb = const.tile([P, DC, 2 * A], F32R)
            ctx_sb = const.tile([P, D], F32R)
            bba_sb = const.tile([P, 3], F32)
            msk_sb = const.tile([1, Tv], I32)
            inp_re = inp_d.ap().rearrange("(o p) d -> p o d", p=P)
            inp_vb = [const.tile([P, D], F32R, name=f"inp{vb}") for vb in range(VB)]
            nc.sync.dma_start(ctx_sb[:], ctx_d.ap())
            nc.sync.dma_start(wkq_sb[:], wkq_d.ap().rearrange("(o p) a -> p o a", p=P))
            for vb in range(4):
                nc.sync.dma_start(inp_vb[vb][:], inp_re[:, vb, :])
            nc.sync.dma_start(bba_sb[:], bba_d.ap())
            for vb in range(4, 8):
                nc.sync.dma_start(inp_vb[vb][:], inp_re[:, vb, :])
            nc.sync.dma_start(msk_sb[:], msk_d.ap())
            bk_sb = bba_sb[:, 0:1]
            bq_sb = bba_sb[:, 1:2]
            av_sb = bba_sb[:, 2:3]
            wk_sb = wkq_sb[:, :, 0:A]
            wq_sb = wkq_sb[:, :, A : 2 * A]

            # ---- small constants ----
            ident = const.tile([P, P], F32)
            make_identity(nc, ident[:])
            ident_r = const.tile([P, P], F32R)
            nc.vector.tensor_copy(ident_r[:], ident[:])

            stage = const.tile([P, 2 * P], F32)
            nc.gpsimd.memset(stage[:], 0.0)
            ones1 = const.tile([1, P], F32R)
            nc.vector.tensor_copy(ones1[:], stage[0:1, 0:P])
            nc.vector.tensor_scalar_add(ones1[:], ones1[:], 1.0)

            # shifted one-hot weights: BIGT[:, 127] = attn_v, else 0
            bigt = const.tile([P, 2 * P - 1], F32R)
            nc.vector.tensor_copy(bigt[:], stage[:, 0 : 2 * P - 1])
            nc.vector.tensor_copy(bigt[:, P - 1 : P], av_sb[:])

            # ---- context transposes + q projection (early; only needs ctx) ----
            ctxT_sb = const.tile([P, DC, P], F32R)
            trc = ps_tr.tile([P, 512], F32R, tag="tr_r")
            for dc in range(DC):
                nc.tensor.transpose(
                    trc[:, dc * P : (dc + 1) * P],
                    ctx_sb[:, dc * P : (dc + 1) * P],
                    ident_r[:],
                )
            nc.vector.tensor_copy(ctxT_sb[:], trc[:])

            # ---- per-half input pipeline + prefill groups (h-major) ----
            # h0 is built in v-quarters (2 blocks each) so the first tanh only
            # waits for the first two input DMAs
            inpT_q = [
                const.tile([P, DC, 256], F32R, name=f"inpTq{i}") for i in range(2)
            ]
            inpT_h1 = const.tile([P, DC, 512], F32R, name="inpTh1")
            kT_q = [const.tile([P, 256], F32, name=f"kTq{i}") for i in range(2)]
            kT_h = [const.tile([P, 512], F32, name=f"kT{h}") for h in range(2)]
            scores_h = [
                ps_sc.tile([P, 512], F32, name=f"scores{h}") for h in range(2)
            ]
            s_pre = [
                spool.tile([P, G, Tv], F32, tag="S", name=f"s_pre{i}")
                for i in range(NPRE)
            ]
            t_pre = [
                tpool.tile([P, G, Tv], F32R, tag="T", name=f"t_pre{i}")
                for i in range(NPRE)
            ]
            # group 0's h0 runs at v-quarter granularity in its own small
            # tiles (separate tiles keep the in-order streams WAR-free)
            s_q = [
                const.tile([P, G, 256], F32, name=f"s_q{i}")
                for i in range(2)
            ]
            # quarter 0's T is half-width with a zeroed pad: its opening
            # matmul must clear the full PSUM bank (start=True zeroes at
            # bank-row granularity, so quarter-width groups cannot interleave)
            t_q = [
                const.tile([P, G, 512 if i == 0 else 256], F32R, name=f"t_q{i}")
                for i in range(2)
            ]
            for j in range(G):
                nc.vector.tensor_copy(t_q[0][:, j, 256:512], stage[:, 0:256])

            def emit_transpose(vb, dst, on_scalar):
                trv = ps_tr.tile([P, 512], F32R, tag="tr_r", name=f"trv{vb}")
                for dc in range(DC):
                    nc.tensor.transpose(
                        trv[:, dc * P : (dc + 1) * P],
                        inp_vb[vb][:, dc * P : (dc + 1) * P],
                        ident_r[:],
                    )
                if on_scalar:
                    nc.scalar.copy(dst, trv[:])
                else:
                    nc.vector.tensor_copy(dst, trv[:])

            def emit_quarter(qtr):
                # quarter pipeline: two v-blocks -> quarter kproj -> kT
                # quarter (for group 0) + assembled kT_h0 region (for the
                # rest)
                for i in range(2):
                    vb = qtr * 2 + i
                    emit_transpose(
                        vb, inpT_q[qtr][:, :, i * P : (i + 1) * P], i % 2 == 0
                    )
                pk = ps_proj.tile([P, 512], F32, tag="proj", name=f"pkq{qtr}")
                for dc in range(DC):
                    nc.tensor.matmul(
                        pk[:, 0:256],
                        wk_sb[:, dc, :],
                        inpT_q[qtr][:, dc, :],
                        start=(dc == 0),
                        stop=(dc == DC - 1),
                    )
                nc.vector.tensor_copy(kT_q[qtr][:], pk[:, 0:256])
                nc.scalar.copy(kT_h[0][:, qtr * 256 : (qtr + 1) * 256], pk[:, 0:256])

            def emit_h1():
                for i in range(4):
                    vb = 4 + i
                    emit_transpose(
                        vb, inpT_h1[:, :, i * P : (i + 1) * P], i % 2 == 0
                    )
                pk = ps_proj.tile([P, 512], F32, tag="proj", name="pkh1")
                for dc in range(DC):
                    nc.tensor.matmul(
                        pk[:],
                        wk_sb[:, dc, :],
                        inpT_h1[:, dc, :],
                        start=(dc == 0),
                        stop=(dc == DC - 1),
                    )
                nc.scalar.copy(kT_h[1][:], pk[:])

            def quarter_tanh_mm(qtr):
                for j in range(G):
                    nc.vector.tensor_scalar_add(
                        s_q[qtr][:, j, :], kT_q[qtr][:], qb_sb[:, j : j + 1]
                    )
                if qtr == 0:
                    nc.scalar.activation(
                        t_q[0][:, :, 0:256], s_q[0][:], AF.Tanh
                    )
                    for j in range(G):
                        nc.tensor.matmul(
                            scores_h[0][:],
                            bigt[:, P - 1 - j : 2 * P - 1 - j],
                            t_q[0][:, j, :],
                            start=(j == 0),
                            stop=False,
                            skip_group_check=True,
                        )
                else:
                    nc.scalar.activation(t_q[1][:], s_q[1][:], AF.Tanh)
                    for j in range(G):
                        nc.tensor.matmul(
                            scores_h[0][:, 256:512],
                            bigt[:, P - 1 - j : 2 * P - 1 - j],
                            t_q[1][:, j, :],
                            start=False,
                            stop=False,
                            skip_group_check=True,
                        )

            def pre_tanh_mm(g, h):
                for j in range(G):
                    nc.vector.tensor_scalar_add(
                        s_pre[g][:, j, h * 512 : (h + 1) * 512],
                        kT_h[h][:],
                        qb_sb[:, g * G + j : g * G + j + 1],
                    )
                nc.scalar.activation(
                    t_pre[g][:, :, h * 512 : (h + 1) * 512],
                    s_pre[g][:, :, h * 512 : (h + 1) * 512],
                    AF.Tanh,
                )
                for j in range(G):
                    q = g * G + j
                    nc.tensor.matmul(
                        scores_h[h][:],
                        bigt[:, P - 1 - q : 2 * P - 1 - q],
                        t_pre[g][:, j, h * 512 : (h + 1) * 512],
                        start=(h == 1 and q == 0),
                        stop=False,
                        skip_group_check=True,
                    )

            qb_sb = const.tile([P, P], F32)

            def emit_qproj():
                bkq_sb = const.tile([P, 1], F32)
                nc.vector.tensor_add(bkq_sb[:], bk_sb[:], bq_sb[:])
                pq = ps_proj.tile([P, P], F32, tag="proj")
                for dc in range(DC):
                    nc.tensor.matmul(
                        pq[:],
                        wq_sb[:, dc, :],
                        ctxT_sb[:, dc, :],
                        start=(dc == 0),
                        stop=(dc == DC - 1),
                    )
                nc.vector.tensor_scalar_add(qb_sb[:], pq[:], bkq_sb[:])

            emit_qproj()
            emit_quarter(0)
            quarter_tanh_mm(0)     # first tanh: only needs v-blocks 0-1
            emit_quarter(1)
            quarter_tanh_mm(1)
            emit_h1()              # second input half flows while tanh runs
            for g in range(1, NPRE):
                pre_tanh_mm(g, 0)
            pre_tanh_mm(0, 1)
            for g in range(1, NPRE):
                pre_tanh_mm(g, 1)

            # mask -> additive row: neg[v] = mask*1e9 - 1e9  (0 if mask==1);
            # emitted here (mask DMA is last, the row is needed only at the
            # end of the score accumulation)
            mskf_sb = const.tile([1, Tv], F32)
            nc.vector.tensor_copy(mskf_sb[:], msk_sb[:])
            neg_sb = const.tile([1, Tv], F32R)
            nc.vector.tensor_scalar(
                neg_sb[:], mskf_sb[:], -NEG_BIG, NEG_BIG,
                mybir.AluOpType.mult, mybir.AluOpType.add,
            )

            # ---- steady-state groups ----
            for g in range(NPRE, NG - 1):
                s_t = spool.tile([P, G, Tv], F32, tag="S")
                for j in range(G):
                    for h in range(2):
                        nc.vector.tensor_scalar_add(
                            s_t[:, j, h * 512 : (h + 1) * 512],
                            kT_h[h][:],
                            qb_sb[:, g * G + j : g * G + j + 1],
                        )
                t_t = tpool.tile([P, G, Tv], F32R, tag="T")
                nc.scalar.activation(t_t[:], s_t[:], AF.Tanh)
                for j in range(G):
                    q = g * G + j
                    for h in range(2):
                        nc.tensor.matmul(
                            scores_h[h][:],
                            bigt[:, P - 1 - q : 2 * P - 1 - q],
                            t_t[:, j, h * 512 : (h + 1) * 512],
                            start=False,
                            stop=False,
                            skip_group_check=True,
                        )

            # ---- last group, h-major, so the h0 softmax/output chain
            # overlaps the h1 tanh; masks interleaved to close each half ----
            gl = NG - 1
            s_l = spool.tile([P, G, Tv], F32, tag="S")
            for j in range(G):
                for h in range(2):
                    nc.vector.tensor_scalar_add(
                        s_l[:, j, h * 512 : (h + 1) * 512],
                        kT_h[h][:],
                        qb_sb[:, gl * G + j : gl * G + j + 1],
                    )
            t_l = tpool.tile([P, G, Tv], F32R, tag="T")
            for h in range(2):
                nc.scalar.activation(
                    t_l[:, :, h * 512 : (h + 1) * 512],
                    s_l[:, :, h * 512 : (h + 1) * 512],
                    AF.Tanh,
                )
                for j in range(G):
                    q = gl * G + j
                    nc.tensor.matmul(
                        scores_h[h][:],
                        bigt[:, P - 1 - q : 2 * P - 1 - q],
                        t_l[:, j, h * 512 : (h + 1) * 512],
                        start=False,
                        stop=False,
                        skip_group_check=True,
                    )
                # additive mask row for this half (rank-1 broadcast), closes
                # the accumulation group so exp can start
                nc.tensor.matmul(
                    scores_h[h][:],
                    ones1[:],
                    neg_sb[:, h * 512 : (h + 1) * 512],
                    start=False,
                    stop=True,
                    skip_group_check=True,
                )

            # ---- softmax over v; raw exp is safe: |scores| <= ||attn_v||_1 ----
            expP_h = [const.tile([P, 512], F32R, name=f"expP{h}") for h in range(2)]
            sumexp_h = const.tile([P, 2], F32)
            for h in range(2):
                nc.scalar.activation(
                    expP_h[h][:],
                    scores_h[h][:],
                    AF.Exp,
                )
                # sumexp on the idle DVE instead of ACT's accum_out: the
                # accum read-back (~190ns each) sits on ACT's serial path
                # right before the P^T transposes
                nc.vector.tensor_reduce(
                    sumexp_h[:, h : h + 1], expP_h[h][:],
                    axis=mybir.AxisListType.X, op=mybir.AluOpType.add,
                )
            sumexp = const.tile([P, 1], F32)
            nc.vector.tensor_reduce(
                sumexp[:], sumexp_h[:], axis=mybir.AxisListType.X,
                op=mybir.AluOpType.add,
            )
            recip = const.tile([P, 1], F32)
            nc.vector.reciprocal(recip[:], sumexp[:])

            # ---- P^T (per half), final matmul, scale ----
            pT_h = [
                const.tile([P, 4, P], F32R, name=f"pT{h}") for h in range(2)
            ]
            po_d = [
                ps_proj.tile([P, 256], F32, tag="proj", name=f"po{dh}")
                for dh in range(2)
            ]
            for half in range(2):
                trp = ps_tr.tile([P, 512], F32R, tag="tr_r")
                for i in range(4):
                    nc.tensor.transpose(
                        trp[:, i * P : (i + 1) * P],
                        expP_h[half][:, i * P : (i + 1) * P],
                        ident_r[:],
                    )
                if half == 0:
                    nc.scalar.copy(pT_h[half][:], trp[:])
                else:
                    nc.vector.tensor_copy(pT_h[half][:], trp[:])
            # accumulate each d-half over all 8 v-blocks; the first d-half's
            # scale + store overlap the second d-half's matmuls
            out_sb = const.tile([P, D], F32)
            for dh in range(2):
                sl = slice(dh * 256, (dh + 1) * 256)
                for vb in range(VB):
                    nc.tensor.matmul(
                        po_d[dh][:],
                        pT_h[vb // 4][:, vb % 4, :],
                        inp_vb[vb][:, sl],
                        start=(vb == 0),
                        stop=(vb == VB - 1),
                    )
                nc.vector.tensor_scalar_mul(out_sb[:, sl], po_d[dh][:], recip[:])
                nc.sync.dma_start(y_d.ap()[:, sl], out_sb[:, sl])

    nc.compile()
    return nc


_NC_CACHE = None


def _get_nc():
    global _NC_CACHE
    if _NC_CACHE is None:
        _NC_CACHE = build_nc()
    return _NC_CACHE


def kernel(inputs, context, mask, Wk, bk, Wq, bq, attn_v):
    nc = _get_nc()
    f32 = np.float32
    wkq = np.concatenate(
        [np.asarray(Wk, dtype=f32), np.asarray(Wq, dtype=f32)], axis=1
    )
    bba = np.stack(
        [np.asarray(bk, f32), np.asarray(bq, f32), np.asarray(attn_v, f32)],
        axis=1,
    )
    in_maps = []
    for c in range(NCORES):
        b, qh = c // 2, c % 2
        in_maps.append({
            "inp": np.ascontiguousarray(inputs[b], dtype=f32),
            "ctx": np.ascontiguousarray(
                context[b, qh * QC : (qh + 1) * QC], dtype=f32
            ),
            "mask": np.ascontiguousarray(mask[b : b + 1, :], dtype=np.int32),
            "wkq": np.ascontiguousarray(wkq),
            "bba": np.ascontiguousarray(bba),
        })
    res = None
    for attempt, delay in enumerate((0, 10, 30)):
        # transient NRT_EXEC_UNIT_UNRECOVERABLE device wedges recover on retry
        if delay:
            time.sleep(delay)
        try:
            res = bass_utils.run_bass_kernel_spmd(
                nc, in_maps, core_ids=list(range(NCORES))
            )
            break
        except Exception:
            if attempt == 2:
                raise
    out = np.empty((B, Tq, D), f32)
    for c in range(NCORES):
        b, qh = c // 2, c % 2
        out[b, qh * QC : (qh + 1) * QC, :] = res.results[c]["y"]
    return out
